# revision 5
# baseline (speedup 1.0000x reference)
"""Trainium2 Bass kernel v2 for nn_DistanceProbeAlternative (retrieval_knn).

Computes, per batch b:
    proj = emb[b] @ W.T                      # [S, R]
    dist[i, j] = ||proj_i||^2 - 2 proj_i . proj_j + ||proj_j||^2

Sharding: data-parallel over batch B=32 across 8 cores (4 batches/core).

v2 redesign vs baseline (117.7us):
  1. Output written fp16 (rel-err budget 2e-2 >> fp16 ~5e-4), cast to fp32
     on host: halves output HBM traffic (16->8 MB/core).
  2. Input loaded as plain fp32 via HWDGE (sync ring): 410+ GB/s measured,
     vs 205 GB/s for the SWDGE cast-DMA path. Cast f32->f16 done on
     DVE (1.23us/quarter) / ACT (2us) / GPSIMD (7us, gets one late quarter).
  3. Epilogue: -2*norms row-vector accumulated into the dots PSUM via a
     K=1 rank-1 matmul (PE has slack at 2 fp16 cols/cycle), so the final
     out = -0.5*psum + ncol is ONE op per [128,512] half:
     DVE tensor_scalar (735ns) or ACT activation bias (686ns), fp16 out.
     (GPSIMD cannot access PSUM; Pool STT fails codegen - measured.)
  4. norms via fp16 sq -> 1-col/row matmuls (cheap fp16 LDW).

Dataflow per batch (W2T16 = -2*W^T fp16, preamble):
  in-DMA fp32 quarters [128,2048] -> cast fp16 -> 64 PE transposes
  -> embT [d, s] -> proj: p2ps[r, 512s] = sum_k W2T16_k^T @ embT_k
  (= -2*projT, f32 PSUM) -> projT2 fp16 + sq16 = Square(0.5*p2ps)
  -> ncol[s,1] = sq16_chunk^T @ ones_col; nrow = ones_col^T @ sq16;
     d_ps seeded with -2*norms via constant(-2) @ sq16 (full-K fp16 MM)
  -> dots: d_ps += projT2_i^T @ projT2 (= 4*dots - 2*nrow)
  -> out = -0.5*d_ps + ncol (DVE TS / ACT bias) -> fp16 out-DMA.
"""

import numpy as np
from contextlib import ExitStack

import concourse.bass as bass
import concourse.bacc as bacc
import concourse.tile as tile
from concourse import mybir
from concourse.bass_utils import run_bass_kernel_spmd
from concourse.masks import make_identity

B, S, D, R = 32, 1024, 1024, 128
NCORES = 8
BPC = B // NCORES  # batches per core
NST = S // 128     # 8 s-tiles
NDT = D // 128     # 8 d-chunks

F32 = mybir.dt.float32
F16 = mybir.dt.float16
IDENT = mybir.ActivationFunctionType.Identity
SQUARE = mybir.ActivationFunctionType.Square
ALU_ADD = mybir.AluOpType.add
ALU_MULT = mybir.AluOpType.mult


def build_nc():
    nc = bacc.Bacc("TRN2", target_bir_lowering=False, debug=False)

    emb = nc.dram_tensor("embeddings_batch", [BPC, S, D], F32, kind="ExternalInput")
    Wd = nc.dram_tensor("W", [R, D], F32, kind="ExternalInput")
    out = nc.dram_tensor("out", [BPC, S, S], F16, kind="ExternalOutput")

    with tile.TileContext(nc) as tc, ExitStack() as ctx:
        constp = ctx.enter_context(tc.tile_pool(name="const", bufs=1))
        esb32_p = ctx.enter_context(tc.tile_pool(name="esb32", bufs=8))
        esb16_p = ctx.enter_context(tc.tile_pool(name="esb16", bufs=8))
        embT_p = ctx.enter_context(tc.tile_pool(name="embT", bufs=3))
        projT2_p = ctx.enter_context(tc.tile_pool(name="projT2", bufs=3))
        sq16_p = ctx.enter_context(tc.tile_pool(name="sq16", bufs=2))
        small_p = ctx.enter_context(tc.tile_pool(name="small", bufs=3))
        out_p = ctx.enter_context(tc.tile_pool(name="outsb", bufs=6))
        tp_p = ctx.enter_context(tc.tile_pool(name="tpsum", bufs=2, space="PSUM"))
        pj_p = ctx.enter_context(tc.tile_pool(name="pjpsum", bufs=1, space="PSUM"))
        dot_p = ctx.enter_context(tc.tile_pool(name="dotps", bufs=4, space="PSUM"))
        nc_p = ctx.enter_context(tc.tile_pool(name="ncps", bufs=1, space="PSUM"))

        # ---------------- constants / W prep ----------------
        def in_dma(b, q, pool_tag="esb32"):
            esb = esb32_p.tile([128, 2048], F32, name=pool_tag)
            src = emb.ap()[b, 256 * q : 256 * (q + 1), :].rearrange(
                "(t p) d -> p t d", p=128
            )
            nc.sync.dma_start(out=esb.rearrange("p (t d) -> p t d", t=2), in_=src)
            return esb

        # ring order: q0 (feeds the first cast/transposes) then W then q1..q3
        esb32 = {}  # (b, q) -> tile
        esb32[(0, 0)] = in_dma(0, 0)
        Wsb32 = constp.tile([128, D], F32, name="Wsb32")
        nc.sync.dma_start(out=Wsb32, in_=Wd.ap())
        for q in range(1, 4):
            esb32[(0, q)] = in_dma(0, q)

        identityf = constp.tile([128, 128], F32, name="identityf")
        make_identity(nc, identityf)
        identity = constp.tile([128, 128], F16, name="identity")
        nc.vector.tensor_copy(identity, identityf)
        onesc = constp.tile([128, 1], F16, name="onesc")  # column of ones [K=r,1]
        nc.gpsimd.memset(onesc, 1.0)
        # all-(-2) stationary: rank-update d_ps = sum_r (-2)*sq[r,:] = -2*norms
        m2s128 = constp.tile([128, 128], F16, name="m2s128")
        nc.gpsimd.memset(m2s128, -2.0)

        # W2 = -2*W in fp16 (DVE: ACT's first-op table-load delay is ~2.6us),
        # then PE-transpose chunks -> W2T16 [d, r]
        W2sb16 = constp.tile([128, D], F16, name="W2sb16")
        nc.vector.tensor_scalar_mul(W2sb16, Wsb32, -2.0)
        W2T16 = constp.tile([128, D], F16, name="W2T16")
        wtp = tp_p.tile([128, 1024], F16, name="tp")
        for k in range(NDT):
            nc.tensor.transpose(
                wtp[:, 128 * k : 128 * (k + 1)],
                W2sb16[:, 128 * k : 128 * (k + 1)],
                identity,
            )
        nc.vector.tensor_copy(W2T16, wtp)

        # ---------------- per-batch phases ----------------
        def cast_quarter(b, q, eng):
            """esb32[b,q] f32 -> esb16 fp16 on the given engine."""
            e16 = esb16_p.tile([128, 2048], F16, name="esb16")
            src = esb32[(b, q)]
            if eng == "v":
                nc.vector.tensor_copy(e16, src)
            elif eng == "a":
                nc.scalar.activation(e16, src, IDENT, bias=0.0, scale=1.0)
            else:
                nc.gpsimd.tensor_copy(e16, src)
            return e16

        # cast engine per quarter: q3 on GPSIMD (needed last, GPSIMD is slow
        # but otherwise idle); q0 DVE (needed first, DVE fastest)
        CAST_ENG = ["v", "a", "v", "g"]

        def trans_quarter(e16, q, embTh):
            """16 PE transposes of one fp16 quarter into embTh [d, s-half].

            embTh is the per-half tile [128, NDT*512]; quarter q covers
            within-half s-tiles si = 2*(q%2)+t.
            """
            embT3 = embTh.rearrange("p (k s) -> p k s", k=NDT)
            for t in range(2):
                si = 2 * (q % 2) + t
                # all 8 d-chunks of one s-tile into ONE psum bank (fp16
                # [128,1024] = 2KB/partition), then a single copy out
                tp = tp_p.tile([128, 1024], F16, name="tp")
                for k in range(NDT):
                    nc.tensor.transpose(
                        tp[:, 128 * k : 128 * (k + 1)],
                        e16[:, 1024 * t + 128 * k : 1024 * t + 128 * (k + 1)],
                        identity,
                    )
                dst = embT3[:, 0:NDT, 128 * si : 128 * (si + 1)]
                tp8 = tp.rearrange("p (k s) -> p k s", k=NDT)
                # split embT copies: DVE 3 of 4, ACT 1 of 4
                if t == 1 and q % 2 == 1:
                    nc.scalar.activation(dst, tp8, IDENT, bias=0.0, scale=1.0)
                else:
                    nc.vector.tensor_copy(dst, tp8)

        def proj_half(h, embTh):
            """p2ps = -2*projT for tokens 512h..512h+511; per-half tiles."""
            p2ps = pj_p.tile([128, 512], F32, name="p2ps")
            for k in range(NDT):
                nc.tensor.matmul(
                    p2ps,
                    W2T16[:, 128 * k : 128 * (k + 1)],
                    embTh[:, 512 * k : 512 * (k + 1)],
                    start=(k == 0),
                    stop=(k == NDT - 1),
                )
            projT2h = projT2_p.tile([128, 512], F16, name="projT2h")
            nc.vector.tensor_copy(projT2h, p2ps)
            sq16h = sq16_p.tile([128, 512], F16, name="sq16h")
            nc.scalar.activation(sq16h, p2ps, SQUARE, bias=0.0, scale=0.5)
            # ncol for the 4 s-tiles of this half: [token, 1] = sum_r proj^2
            ncol_ps = nc_p.tile([128, 4], F32, name="ncol_ps")
            for tt in range(4):
                nc.tensor.matmul(
                    ncol_ps[:, tt : tt + 1],
                    sq16h[:, 128 * tt : 128 * (tt + 1)],
                    onesc,
                    start=True,
                    stop=True,
                )
            ncolh = small_p.tile([128, 4], F32, name="ncolh")
            nc.vector.tensor_copy(ncolh, ncol_ps)
            return projT2h, ncolh, sq16h

        def dots_pair_h(b, pair, h, projT2h, ncolh, sq16h, eng=None):
            """Two i-tiles (2*pair, 2*pair+1), half h -> outsb -> DMA.

            d_ps is seeded with -2*norms_j via a full-K matmul of the
            constant (-2) stationary against sq16h (1 fp16-rate MM, no
            nrow row-vector needed), then the dots accumulate on top.
            Epilogue engine alternates by pair so each outsb tile has a
            single writer (avoids cross-engine ordering on shared tiles).
            """
            outsb = out_p.tile([128, 1024], F16, name="outsb")
            if eng is None:
                # 10 halves on ACT, 6 on DVE (DVE also carries casts+copies)
                eng = "v" if (pair, h) in ((1, 0), (3, 0), (1, 1)) else "a"
            for t in range(2):
                i = 2 * pair + t
                d_ps = dot_p.tile([128, 512], F32, name="d_ps")
                nc.tensor.matmul(d_ps, m2s128, sq16h, start=True, stop=False)
                nc.tensor.matmul(
                    d_ps,
                    projT2h[i // 4][:, 128 * (i % 4) : 128 * (i % 4 + 1)],
                    projT2h[h],
                    start=False,
                    stop=True,
                )
                dst = outsb[:, 512 * t : 512 * (t + 1)]
                nci = ncolh[i // 4][:, i % 4 : i % 4 + 1]
                if eng == "v":
                    nc.vector.tensor_scalar(
                        out=dst, in0=d_ps, scalar1=-0.5, scalar2=nci,
                        op0=ALU_MULT, op1=ALU_ADD,
                    )
                else:
                    nc.scalar.activation(dst, d_ps, IDENT, bias=nci, scale=-0.5)
            dram_dst = out.ap()[
                b, 256 * pair : 256 * (pair + 1), 512 * h : 512 * (h + 1)
            ].rearrange("(t p) x -> p t x", p=128)
            nc.sync.dma_start(
                out=dram_dst,
                in_=outsb.rearrange("p (t x) -> p t x", t=2),
            )

        # ---------------- batch pipeline (explicit software pipeline) ----
        # dots order within a batch: j -> (pair, h)
        DOTS_J = [(0, 0), (1, 0), (2, 0), (3, 0), (0, 1), (1, 1), (2, 1), (3, 1)]

        def dots_j(b, j, st):
            pair, h = DOTS_J[j]
            dots_pair_h(b, pair, h, st["projT2h"], st["ncolh"], st["sq16h"][h])

        def proj_and_first_dots(b, st, e16):
            """trans q0,q1 + proj h0 + dots j=0,1 for batch b (head bricks)."""
            embTh = embT_p.tile([128, NDT * 512], F16, name="embTh")
            trans_quarter(e16[0], 0, embTh)
            trans_quarter(e16[1], 1, embTh)
            pj, ncl, sqh = proj_half(0, embTh)
            st["projT2h"].append(pj)
            st["ncolh"].append(ncl)
            st["sq16h"].append(sqh)
            dots_j(b, 0, st)
            dots_j(b, 1, st)

        def proj_second_half(b, st, e16):
            embTh = embT_p.tile([128, NDT * 512], F16, name="embTh")
            trans_quarter(e16[2], 2, embTh)
            trans_quarter(e16[3], 3, embTh)
            pj, ncl, sqh = proj_half(1, embTh)
            st["projT2h"].append(pj)
            st["ncolh"].append(ncl)
            st["sq16h"].append(sqh)

        # ---- head: batch 0 ----
        e16 = {q: cast_quarter(0, q, "v" if q % 2 == 0 else "a") for q in range(4)}
        st = {"projT2h": [], "ncolh": [], "sq16h": []}
        proj_and_first_dots(0, st, e16)
        esb32[(1, 0)] = in_dma(1, 0)
        esb32[(1, 1)] = in_dma(1, 1)
        proj_second_half(0, st, e16)
        esb32[(1, 2)] = in_dma(1, 2)
        esb32[(1, 3)] = in_dma(1, 3)
        e16_next = {
            0: cast_quarter(1, 0, CAST_ENG[0]),
            3: cast_quarter(1, 3, CAST_ENG[3]),
        }

        # ---- steady loop: emit dots(b-1) j=2..7 interleaved with batch-b
        # transposes/proj and batch-(b+1) input DMAs + casts ----
        for b in range(1, BPC):
            prev_st, prev_e16 = st, e16
            e16 = e16_next
            st = {"projT2h": [], "ncolh": [], "sq16h": []}
            nxt = b + 1 < BPC

            dots_j(b - 1, 2, prev_st)
            e16[1] = cast_quarter(b, 1, CAST_ENG[1])
            dots_j(b - 1, 3, prev_st)
            dots_j(b - 1, 4, prev_st)
            st_embT0 = embT_p.tile([128, NDT * 512], F16, name="embTh")
            trans_quarter(e16[0], 0, st_embT0)
            dots_j(b - 1, 5, prev_st)
            e16[2] = cast_quarter(b, 2, CAST_ENG[2])
            dots_j(b - 1, 6, prev_st)
            trans_quarter(e16[1], 1, st_embT0)
            dots_j(b - 1, 7, prev_st)
            pj, ncl, sqh = proj_half(0, st_embT0)
            st["projT2h"].append(pj)
            st["ncolh"].append(ncl)
            st["sq16h"].append(sqh)
            dots_j(b, 0, st)
            dots_j(b, 1, st)
            if nxt:
                esb32[(b + 1, 0)] = in_dma(b + 1, 0)
                esb32[(b + 1, 1)] = in_dma(b + 1, 1)
            proj_second_half(b, st, e16)
            if nxt:
                esb32[(b + 1, 2)] = in_dma(b + 1, 2)
                esb32[(b + 1, 3)] = in_dma(b + 1, 3)
                e16_next = {
                    0: cast_quarter(b + 1, 0, CAST_ENG[0]),
                    3: cast_quarter(b + 1, 3, CAST_ENG[3]),
                }

        # ---- tail: dots j=2..7 of the last batch; alternate the epilogue
        # engine per tile so the final drain pipelines across ACT/DVE ----
        for jj, j in enumerate(range(2, 8)):
            pair, h = DOTS_J[j]
            dots_pair_h(
                BPC - 1, pair, h, st["projT2h"], st["ncolh"], st["sq16h"][h],
                eng="a" if jj % 2 == 0 else "v",
            )

    nc.finalize()
    return nc


_NC_CACHE = None


def _get_nc():
    global _NC_CACHE
    if _NC_CACHE is None:
        _NC_CACHE = build_nc()
    return _NC_CACHE


def run(embeddings_batch, W, trace=False, tmpdir=None):
    nc = _get_nc()
    emb = np.asarray(embeddings_batch, dtype=np.float32)
    Wf = np.ascontiguousarray(np.asarray(W, dtype=np.float32))
    in_maps = [
        {
            "embeddings_batch": np.ascontiguousarray(emb[c * BPC : (c + 1) * BPC]),
            "W": Wf,
        }
        for c in range(NCORES)
    ]
    res = run_bass_kernel_spmd(
        nc, in_maps, core_ids=list(range(NCORES)), trace=trace, tmpdir=tmpdir
    )
    full = np.concatenate(
        [r["out"].astype(np.float32) for r in res.results], axis=0
    )
    return full, res


def kernel(embeddings_batch, W):
    full, _ = run(embeddings_batch, W, trace=False)
    return full
